# revision 19
# baseline (speedup 1.0000x reference)
"""GumbelGraphNetworkClf fused Bass kernel for 8 trn2 NeuronCores (raw bass).

Math (per batch b):
  pre[i,j,:] = x[j]@W_e1[:D] + x[i]@W_e1[D:] + b_e1   (= A[j] + C[i])
  n2e = relu(pre); e2e = relu(n2e @ W_e2 + b_e2)
  agg[j,:] = sum_i adj[i,j] * e2e[i,j,:]
  out = log_softmax(nodeMLP(agg, x), axis=-1)

Sharding: core c -> batch b = c//2, i-half = c%2.  Each core runs the edge
pipeline for its own i rows 0..239 over all 512 j (iterations 0..239), in
two accumulation phases (i 0..127, 128..239).  Each phase's partial agg is
pair-ReduceScattered over the j axis (rank = c%2 keeps its j-half); phase
1's collective hides under phase 2, phase 2's under a replicated tail
phase: both pair cores compute the 32 leftover global i rows ({240..255,
496..511}) for their own j-half at FD=256 (iterations 240..271).  The node
MLP + log_softmax then run on [*, 256] tiles and each core writes its half
of the output rows.

Per iteration it (fd = 512 main / 256 tail):
  DVE  n2e(it+4) = max(4A + 4C[:,it+4], 0) -> fp8     (2 tensor_scalar)
  PE   py(it)    = W2q.T @ n2e                        (2 fp8 DoubleRow MM)
       acc(it-9) agg += I @ msk(it-9)                 (2 f16 MM)
  ACT  e2e(it)   = relu(py/128 + b2) -> f16           (2 activation)
  DVE  msk quad  = e2e[4i block] * abc                (2 tensor_mul / 4 it)
W2 is sent as fp8(32*W2) and n2e is scaled by 4 into fp8's normal range
(the 1/128 comes out in the ACT scale), so the 256-deep contraction runs
as one DoubleRow matmul per output chunk.
"""

import sys

sys.path.insert(0, "/opt/trn_rl_repo")

import numpy as np
import ml_dtypes

import concourse.bass as bass
from concourse import mybir
from concourse.bass_utils import run_bass_kernel_spmd

B, N, D, H = 4, 512, 4, 256
NI = 256          # i rows per core's shard
NM = 240          # main-loop iterations (own i rows 0..239)
NP3 = 32          # replicated tail rows ({240..255, 496..511})
NT = NM + NP3     # total pipeline iterations
NH = N // 2       # j-half kept after ReduceScatter
HT = 2
LAG = 9           # acc(it-LAG) in PE iteration it
F16 = mybir.dt.float16
F32 = mybir.dt.float32
F8 = mybir.dt.float8e4
AF = mybir.ActivationFunctionType
OP = mybir.AluOpType
PM = mybir.MatmulPerfMode

# f16 const pack (cols) — A/A3/ident first (early DMA slice)
O_A = 0                    # 4*A^T halves          [128, 2N]
O_A3 = O_A + 2 * N         # 4*A^T j-half          [128, 2NH]
O_ID = O_A3 + 2 * NH       # identity              [128, 128]
NC16 = O_ID + 128          # end of early slice
O_WN1 = NC16               # W_n1 chunks           [128, 2H]
O_WN2 = O_WN1 + 2 * H
O_WO1H = O_WN2 + 2 * H
O_WO = O_WO1H + 2 * H      # W_o chunks            [128, 2D]
O_XT = O_WO + 2 * D        # x^T j-half            rows 0-3, NH cols
O_WO1X = O_XT + NH         # W_o1[:D]              rows 0-3, H cols
O_ONES4 = O_WO1X + H       # [4,1] ones
O_ONES14 = O_ONES4 + 1     # [1,4] ones
CF16 = O_ONES14 + 4
# f32 const pack (cols)
O_C = 0                    # 4*C^T own rows        [128, 2NM]
O_C3 = O_C + 2 * NM        # 4*C^T tail rows       [128, 2NP3]
O_B2 = O_C3 + 2 * NP3
O_BN1 = O_B2 + 2
O_BN2 = O_BN1 + 2
O_BO1 = O_BN2 + 2
O_BO = O_BO1 + 2           # rows 0-3
CF32 = O_BO + 1

_CACHE = {}


def _fd(it):
    return N if it < NM else NH


def _ph(k):
    return 0 if k < 128 else (1 if k < NM else 0)


def build_program():
    nc = bass.Bass("TRN2", target_bir_lowering=False, num_devices=8)

    cf16_ext = nc.dram_tensor("cf16", [128, CF16], F16, kind="ExternalInput")
    cf32_ext = nc.dram_tensor("cf32", [128, CF32], F32, kind="ExternalInput")
    w28_ext = nc.dram_tensor("w28", [128, 2, H], F8, kind="ExternalInput")
    adj_ext = nc.dram_tensor("adjr", [NM, N], F16, kind="ExternalInput")
    adj3_ext = nc.dram_tensor("adj3", [NP3, NH], F16, kind="ExternalInput")
    out_ext = nc.dram_tensor("out", [D, NH], F32, kind="ExternalOutput")
    aggd = [nc.dram_tensor(f"aggd{p}", [2, 128, 2, NH], F16) for p in range(2)]
    aggr = [nc.dram_tensor(f"aggr{p}", [128, 2, NH], F16) for p in range(2)]

    IT_COPY1 = 158      # ACT inserts phase-1 agg copies after this iteration
    IT_COPY2 = 250      # DVE inserts phase-2 agg copies after this iteration

    # ================= milestone bookkeeping (mirrors emission) =============
    # v_sem (DVE): prologue n2e(0..3); loop it: [n2e(it+4)], [quad at
    # it%4==3], [copy2 at IT_COPY2].
    vcnt = 0
    n2e_done, mskq_done_ho, mskq_done = {}, {}, {}
    V_COPY2 = None
    for k in range(4):
        vcnt += 2
        n2e_done[k] = vcnt
    for it in range(NT):
        if it + 4 < NT:
            vcnt += 2
            n2e_done[it + 4] = vcnt
        if it % 4 == 3:
            Q = it // 4
            vcnt += 1
            mskq_done_ho[(Q, 0)] = vcnt
            vcnt += 1
            mskq_done_ho[(Q, 1)] = vcnt
            mskq_done[Q] = vcnt
        if it == IT_COPY2:
            vcnt += 2
            V_COPY2 = vcnt

    # p_sem (PE): loop it: py(it) per ho (1 inc each); acc(it-LAG) (2 incs).
    pcnt = 0
    py_done_ho, py_done, acc_done = {}, {}, {}
    for it in range(NT + LAG):
        if it < NT:
            pcnt += 1
            py_done_ho[(it, 0)] = pcnt
            pcnt += 1
            py_done_ho[(it, 1)] = pcnt
            py_done[it] = pcnt
        if it >= LAG:
            pcnt += 2
            acc_done[it - LAG] = pcnt

    # a_sem (ACT): e2e per (it, ho); copy1 inserted after IT_COPY1.
    acnt = 0
    e2e_done_ho, e2e_done = {}, {}
    A_COPY1 = None
    for it in range(NT):
        for ho in range(HT):
            acnt += 1
            e2e_done_ho[(it, ho)] = acnt
        e2e_done[it] = acnt
        if it == IT_COPY1:
            acnt += 2
            A_COPY1 = acnt
    A_AGG3C = acnt + 2

    # d_sem: each dma incs 16.  Order: cf32, cf16-A, w28, cf16-A3/ident,
    # abc0, abc1, cf16rest, abc2..19, aggd1(4), abc20..22, abc3big,
    # abc23..29, aggd2(4), rs1 load, rs2 load, out.
    dcnt = 0
    def _dma():
        nonlocal dcnt
        dcnt += 16
        return dcnt
    for _ in range(3):
        D_CORE = _dma()
    abc_d = {}
    for k in range(2):
        abc_d[k] = _dma()
    D_REST = _dma()
    for k in range(2, 20):
        abc_d[k] = _dma()
    for _ in range(4):
        D_AGGD1 = _dma()
    for k in range(20, 23):
        abc_d[k] = _dma()
    D_ABC3 = _dma()
    for k in range(23, NM // 8):
        abc_d[k] = _dma()
    for _ in range(4):
        D_AGGD2 = _dma()
    D_R1 = _dma()
    D_R2 = _dma()

    def d_abc(it):
        return abc_d[it // 8] if it < NM else D_ABC3

    from contextlib import ExitStack
    with ExitStack() as ctx:
        e = ctx.enter_context
        cf16 = e(nc.sbuf_tensor([128, CF16], F16))
        cf32 = e(nc.sbuf_tensor([128, CF32], F32))
        w28 = e(nc.sbuf_tensor([128, 2, H], F8))
        abc = [e(nc.sbuf_tensor(f"abc{k}", [128, 8 * N], F16)) for k in range(2)]
        abc3 = e(nc.sbuf_tensor("abc3", [128, NP3 * NH], F16))
        n2e = [e(nc.sbuf_tensor(f"n2e{q}", [128, 2, N], F8)) for q in range(6)]
        e2eq = [[e(nc.sbuf_tensor(f"e2eq{t}{r}", [128, 4 * N], F16)) for r in range(3)] for t in range(2)]
        mskq = [[e(nc.sbuf_tensor(f"mskq{t}{r}", [128, 4 * N], F16)) for r in range(3)] for t in range(2)]
        aggf = [[e(nc.sbuf_tensor(f"aggf{p}{k}", [128, N], F16)) for k in range(2)] for p in range(2)]
        agg2 = [e(nc.sbuf_tensor(f"agg2{p}", [128, 2, NH], F16)) for p in range(2)]
        agg3sb = [e(nc.sbuf_tensor(f"agg3sb{k}", [128, NH], F16)) for k in range(2)]
        out1 = [e(nc.sbuf_tensor(f"out1{k}", [128, NH], F16)) for k in range(2)]
        out2 = [e(nc.sbuf_tensor(f"out2{k}", [128, NH], F16)) for k in range(2)]
        out4 = [e(nc.sbuf_tensor(f"out4{k}", [128, NH], F16)) for k in range(2)]
        out5 = e(nc.sbuf_tensor([4, NH], F32))
        ex = e(nc.sbuf_tensor([4, NH], F16))
        ls = e(nc.sbuf_tensor([1, NH], F16))
        res = e(nc.sbuf_tensor([4, NH], F32))
        dumm = e(nc.sbuf_tensor([1, 2], F32))
        py00 = e(nc.psum_tensor([128, N], F32))
        py01 = e(nc.psum_tensor([128, N], F32))
        py10 = e(nc.psum_tensor([128, N], F32))
        py11 = e(nc.psum_tensor([128, N], F32))
        agg00 = e(nc.psum_tensor([128, N], F32))
        agg01 = e(nc.psum_tensor([128, N], F32))
        agg10 = e(nc.psum_tensor([128, N], F32))
        agg11 = e(nc.psum_tensor([128, N], F32))
        d_sem = e(nc.semaphore("d_sem"))
        v_sem = e(nc.semaphore("v_sem"))
        p_sem = e(nc.semaphore("p_sem"))
        a_sem = e(nc.semaphore("a_sem"))
        cc_sem = e(nc.semaphore("cc_sem"))
        pp_sem = e(nc.semaphore("pp_sem"))
        aa_sem = e(nc.semaphore("aa_sem"))
        vv_sem = e(nc.semaphore("vv_sem"))
        block = e(nc.Block())
        py = [[py00, py01], [py10, py11]]
        agg_ps = [[agg00, agg01], [agg10, agg11]]     # [phase][ho-chunk]
        A_sb = [cf16[:, O_A + t * N : O_A + (t + 1) * N] for t in range(HT)]
        A3_sb = [cf16[:, O_A3 + t * NH : O_A3 + (t + 1) * NH] for t in range(HT)]
        ident = cf16[:, O_ID : O_ID + 128]
        Wn1_sb = [cf16[:, O_WN1 + t * H : O_WN1 + (t + 1) * H] for t in range(HT)]
        Wn2_sb = [cf16[:, O_WN2 + t * H : O_WN2 + (t + 1) * H] for t in range(HT)]
        Wo1h_sb = [cf16[:, O_WO1H + t * H : O_WO1H + (t + 1) * H] for t in range(HT)]
        Wo_sb = [cf16[:, O_WO + t * D : O_WO + (t + 1) * D] for t in range(HT)]
        xT_sb = cf16[0:D, O_XT : O_XT + NH]
        Wo1x_sb = cf16[0:D, O_WO1X : O_WO1X + H]
        ones4 = cf16[0:D, O_ONES4 : O_ONES4 + 1]
        ones14 = cf16[0:1, O_ONES14 : O_ONES14 + 4]
        C_sb = [cf32[:, O_C + t * NM : O_C + (t + 1) * NM] for t in range(HT)]
        C3_sb = [cf32[:, O_C3 + t * NP3 : O_C3 + (t + 1) * NP3] for t in range(HT)]
        b2_sb = [cf32[:, O_B2 + t : O_B2 + t + 1] for t in range(HT)]
        bn1_sb = [cf32[:, O_BN1 + t : O_BN1 + t + 1] for t in range(HT)]
        bn2_sb = [cf32[:, O_BN2 + t : O_BN2 + t + 1] for t in range(HT)]
        bo1_sb = [cf32[:, O_BO1 + t : O_BO1 + t + 1] for t in range(HT)]
        bo_sb = cf32[0:D, O_BO : O_BO + 1]
        # post-loop PSUM reuse (agg10/11 free after copy2; py banks after loop)
        ps_mlp = [agg10[:, 0:NH], agg11[:, 0:NH]]
        ps5 = py00[0:4, 0:NH]
        ps_sum = py01[0:1, 0:NH]
        ps_ls4 = py11[0:4, 0:NH]

        def n2e_src(k, t):
            if k < NM:
                return A_sb[t], C_sb[t][:, k : k + 1]
            return A3_sb[t], C3_sb[t][:, k - NM : k - NM + 1]

        @block.sync
        def _(sync):
            sync.dma_start(cf32[:], cf32_ext[:, :]).then_inc(d_sem, 16)
            sync.dma_start(cf16[:, 0:NC16], cf16_ext[:, 0:NC16]).then_inc(d_sem, 16)
            sync.dma_start(w28[:, :, :], w28_ext[:, :, :]).then_inc(d_sem, 16)
            for k in range(NM // 8):
                if k >= 2:
                    sync.wait_ge(v_sem, mskq_done[2 * (k - 2) + 1])  # abc WAR
                sync.dma_start(
                    abc[k % 2][:],
                    adj_ext[None, 8 * k : 8 * (k + 1), :].broadcast_to([128, 8, N]),
                ).then_inc(d_sem, 16)
                if k == 1:
                    sync.dma_start(cf16[:, NC16:CF16], cf16_ext[:, NC16:CF16]).then_inc(d_sem, 16)
                if k == 19:
                    sync.wait_ge(a_sem, A_COPY1)
                    for half in range(2):
                        for t in range(HT):
                            sync.dma_start(
                                aggd[0][half, :, t, :],
                                aggf[0][t][:, half * NH : (half + 1) * NH],
                            ).then_inc(d_sem, 16)
                if k == 22:
                    sync.dma_start(
                        abc3[:],
                        adj3_ext[None, :, :].broadcast_to([128, NP3, NH]),
                    ).then_inc(d_sem, 16)
            sync.wait_ge(v_sem, V_COPY2)
            for half in range(2):
                for t in range(HT):
                    sync.dma_start(
                        aggd[1][half, :, t, :],
                        aggf[1][t][:, half * NH : (half + 1) * NH],
                    ).then_inc(d_sem, 16)
            sync.wait_ge(cc_sem, 1)
            sync.dma_start(agg2[0][:, :, :], aggr[0][:, :, :]).then_inc(d_sem, 16)
            sync.wait_ge(cc_sem, 2)
            sync.dma_start(agg2[1][:, :, :], aggr[1][:, :, :]).then_inc(d_sem, 16)
            sync.wait_ge(vv_sem, 1)
            sync.dma_start(out_ext[:, :], res[:, :]).then_inc(d_sem, 16)

        @block.gpsimd
        def _(gpsimd):
            for p in range(2):
                gpsimd.wait_ge(d_sem, D_AGGD1 if p == 0 else D_AGGD2)
                nc.gpsimd.collective_compute(
                    "ReduceScatter", OP.add,
                    replica_groups=[[0, 1], [2, 3], [4, 5], [6, 7]],
                    ins=[aggd[p][:]], outs=[aggr[p][:]],
                ).then_inc(cc_sem, 1)

        @block.vector
        def _(vector):
            vector.wait_ge(d_sem, D_CORE)
            for k in range(4):
                for t in range(HT):
                    asrc, csrc = n2e_src(k, t)
                    nc.vector.tensor_scalar(
                        n2e[k % 6][:, t, 0:_fd(k)], asrc, csrc, 0.0,
                        op0=OP.add, op1=OP.max,
                    ).then_inc(v_sem, 1)
            for it in range(NT):
                if it + 4 < NT:
                    if it >= 2:
                        vector.wait_ge(p_sem, py_done[it - 2])  # n2e slot WAR
                    for t in range(HT):
                        asrc, csrc = n2e_src(it + 4, t)
                        nc.vector.tensor_scalar(
                            n2e[(it + 4) % 6][:, t, 0:_fd(it + 4)], asrc, csrc, 0.0,
                            op0=OP.add, op1=OP.max,
                        ).then_inc(v_sem, 1)
                if it % 4 == 3:
                    Q = it // 4
                    fd = _fd(it)
                    vector.wait_ge(d_sem, d_abc(it))
                    if Q >= 3:
                        vector.wait_ge(p_sem, acc_done[4 * (Q - 3) + 3])  # mskq WAR
                    if it < NM:
                        src1 = abc[(it // 8) % 2][:, (Q % 2) * 4 * N : (Q % 2 + 1) * 4 * N]
                    else:
                        q3 = Q - NM // 4
                        src1 = abc3[:, q3 * 4 * NH : (q3 + 1) * 4 * NH]
                    for ho in range(HT):
                        vector.wait_ge(a_sem, e2e_done_ho[(it, ho)])
                        nc.vector.tensor_mul(
                            mskq[ho][Q % 3][:, 0 : 4 * fd],
                            e2eq[ho][Q % 3][:, 0 : 4 * fd], src1,
                        ).then_inc(v_sem, 1)
                if it == IT_COPY2:
                    vector.wait_ge(p_sem, acc_done[NM - 1])
                    for t in range(HT):
                        nc.vector.tensor_copy(aggf[1][t][:], agg_ps[1][t][:]).then_inc(v_sem, 1)
            vector.wait_ge(pp_sem, 9)
            nc.vector.tensor_tensor(res[:], out5[:], ps_ls4, op=OP.subtract).then_inc(vv_sem, 1)

        @block.scalar
        def _(scalar):
            # activation-table preload: force the natural_log_exp set early
            nc.scalar.activation(dumm[:, 0:1], dumm[:, 1:2], AF.Ln)
            nc.scalar.activation(dumm[:, 0:1], dumm[:, 1:2], AF.Exp)
            for it in range(NT):
                fd = _fd(it)
                for ho in range(HT):
                    scalar.wait_ge(p_sem, py_done_ho[(it, ho)])
                    if it >= 12:
                        # e2eq[ho][(it//4)%3] slot WAR vs mskq of quad it//4-3
                        scalar.wait_ge(v_sem, mskq_done_ho[(it // 4 - 3, ho)])
                    nc.scalar.activation(
                        e2eq[ho][(it // 4) % 3][:, (it % 4) * fd : (it % 4 + 1) * fd],
                        py[ho][it % 2][:, 0:fd], AF.Relu, bias=b2_sb[ho], scale=0.0078125,
                    ).then_inc(a_sem, 1)
                if it == IT_COPY1:
                    scalar.wait_ge(p_sem, acc_done[127])
                    for t in range(HT):
                        nc.scalar.activation(aggf[0][t][:], agg_ps[0][t][:],
                                             AF.Identity).then_inc(a_sem, 1)
            scalar.wait_ge(p_sem, acc_done[NT - 1])
            for t in range(HT):
                nc.scalar.activation(agg3sb[t][:], agg_ps[0][t][:, 0:NH],
                                     AF.Identity).then_inc(a_sem, 1)
            # ---- node MLP activations ----
            for ho in range(HT):
                scalar.wait_ge(pp_sem, ho + 1)
                nc.scalar.activation(out1[ho][:], ps_mlp[ho], AF.Relu,
                                     bias=bn1_sb[ho]).then_inc(aa_sem, 1)
            for ho in range(HT):
                scalar.wait_ge(pp_sem, 2 + ho + 1)
                nc.scalar.activation(out2[ho][:], ps_mlp[ho], AF.Relu,
                                     bias=bn2_sb[ho]).then_inc(aa_sem, 1)
            for ho in range(HT):
                scalar.wait_ge(pp_sem, 4 + ho + 1)
                nc.scalar.activation(out4[ho][:], ps_mlp[ho], AF.Identity,
                                     bias=bo1_sb[ho]).then_inc(aa_sem, 1)
            scalar.wait_ge(pp_sem, 7)
            nc.scalar.activation(ex[:], ps5, AF.Exp, bias=bo_sb).then_inc(aa_sem, 1)
            nc.scalar.activation(out5[:], ps5, AF.Identity, bias=bo_sb).then_inc(aa_sem, 1)
            scalar.wait_ge(pp_sem, 8)
            nc.scalar.activation(ls[:], ps_sum, AF.Ln).then_inc(aa_sem, 1)

        @block.tensor
        def _(pe):
            for it in range(NT + LAG):
                if it < NT:
                    fd = _fd(it)
                    q = it % 2
                    pe.wait_ge(v_sem, n2e_done[it])
                    for ho in range(HT):
                        if it >= 2:
                            pe.wait_ge(a_sem, e2e_done_ho[(it - 2, ho)])  # py WAR
                        hs = slice(ho * 128, (ho + 1) * 128)
                        nc.tensor.matmul(
                            py[ho][q][:, 0:fd], w28[:, 0:2, hs], n2e[it % 6][:, :, 0:fd],
                            start=True, stop=True, perf_mode=PM.DoubleRow,
                        ).then_inc(p_sem, 1)
                if it >= LAG:
                    k = it - LAG
                    fdk = _fd(k)
                    ph = _ph(k)
                    pe.wait_ge(v_sem, mskq_done[k // 4])
                    for ho in range(HT):
                        nc.tensor.matmul(
                            agg_ps[ph][ho][:, 0:fdk], ident,
                            mskq[ho][(k // 4) % 3][:, (k % 4) * fdk : (k % 4 + 1) * fdk],
                            start=(k in (0, 128, NM)), stop=(k in (127, NM - 1, NT - 1)),
                        ).then_inc(p_sem, 1)
            # ---- node MLP matmuls ----  pp milestones:
            # 1,2: mlp1[ho]  3,4: mlp2[ho]  5,6: out4 ps[ho]  7: ps5
            # 8: ps_sum  9: ps_ls4
            pe.wait_ge(d_sem, D_REST)
            pe.wait_ge(v_sem, V_COPY2)
            pe.wait_ge(d_sem, D_R1)
            for ho in range(HT):
                for t in range(HT):
                    nc.tensor.matmul(
                        ps_mlp[ho], Wn1_sb[t][:, ho * 128 : (ho + 1) * 128],
                        agg2[0][:, t, :], start=(t == 0), stop=False,
                    )
            pe.wait_ge(a_sem, A_AGG3C)
            for ho in range(HT):
                for t in range(HT):
                    nc.tensor.matmul(
                        ps_mlp[ho], Wn1_sb[t][:, ho * 128 : (ho + 1) * 128],
                        agg3sb[t][:], start=False, stop=False,
                    )
            pe.wait_ge(d_sem, D_R2)
            for ho in range(HT):
                for t in range(HT):
                    mm = nc.tensor.matmul(
                        ps_mlp[ho], Wn1_sb[t][:, ho * 128 : (ho + 1) * 128],
                        agg2[1][:, t, :], start=False, stop=(t == 1),
                    )
                mm.then_inc(pp_sem, 1)
            for ho in range(HT):
                pe.wait_ge(aa_sem, 2)
                for t in range(HT):
                    mm = nc.tensor.matmul(
                        ps_mlp[ho], Wn2_sb[t][:, ho * 128 : (ho + 1) * 128],
                        out1[t][:], start=(t == 0), stop=(t == 1),
                    )
                mm.then_inc(pp_sem, 1)
            for ho in range(HT):
                pe.wait_ge(aa_sem, 4)
                nc.tensor.matmul(
                    ps_mlp[ho], Wo1x_sb[:, ho * 128 : (ho + 1) * 128], xT_sb,
                    start=True, stop=False,
                )
                for t in range(HT):
                    mm = nc.tensor.matmul(
                        ps_mlp[ho], Wo1h_sb[t][:, ho * 128 : (ho + 1) * 128],
                        out2[t][:], start=False, stop=(t == 1),
                    )
                mm.then_inc(pp_sem, 1)
            pe.wait_ge(aa_sem, 6)
            for t in range(HT):
                mm = nc.tensor.matmul(
                    ps5, Wo_sb[t], out4[t][:], start=(t == 0), stop=(t == 1),
                )
            mm.then_inc(pp_sem, 1)
            pe.wait_ge(aa_sem, 7)                             # ex ready
            nc.tensor.matmul(ps_sum, ones4, ex[:], start=True, stop=True).then_inc(pp_sem, 1)
            pe.wait_ge(aa_sem, 9)                             # ls ready
            nc.tensor.matmul(ps_ls4, ones14, ls[:], start=True, stop=True).then_inc(pp_sem, 1)

    return nc


P3_ROWS = list(range(NM, NI)) + list(range(NI + NM, 2 * NI))


def make_in_maps(x, adj, W_e1, b_e1, W_e2, b_e2, W_n1, b_n1, W_n2, b_n2,
                 W_o1, b_o1, W_o, b_o):
    F8NP = ml_dtypes.float8_e4m3
    w28 = np.zeros((128, 2, H), F8NP)
    w2s = 32.0 * W_e2.astype(np.float32)
    for t in range(HT):
        w28[:, t, :] = w2s[t * 128 : (t + 1) * 128].astype(F8NP)

    in_maps = []
    for c in range(8):
        b = c // 2
        half = c % 2
        i0 = half * NI
        A_full = x[b] @ W_e1[:D]                     # [N, H]
        C_full = x[b] @ W_e1[D:] + b_e1              # [N, H]

        cf16 = np.zeros((128, CF16), np.float16)
        AT = (4.0 * A_full.T).astype(np.float16)     # [H, N]
        for t in range(HT):
            r = slice(t * 128, (t + 1) * 128)
            cf16[:, O_A + t * N : O_A + (t + 1) * N] = AT[r]
            cf16[:, O_A3 + t * NH : O_A3 + (t + 1) * NH] = AT[r][:, half * NH : (half + 1) * NH]
            cf16[:, O_WN1 + t * H : O_WN1 + (t + 1) * H] = W_n1[r].astype(np.float16)
            cf16[:, O_WN2 + t * H : O_WN2 + (t + 1) * H] = W_n2[r].astype(np.float16)
            cf16[:, O_WO1H + t * H : O_WO1H + (t + 1) * H] = W_o1[D:][r].astype(np.float16)
            cf16[:, O_WO + t * D : O_WO + (t + 1) * D] = W_o[r].astype(np.float16)
        cf16[:, O_ID : O_ID + 128] = np.eye(128, dtype=np.float16)
        cf16[0:D, O_XT : O_XT + NH] = x[b].T[:, half * NH : (half + 1) * NH]
        cf16[0:D, O_WO1X : O_WO1X + H] = W_o1[:D]
        cf16[0:D, O_ONES4] = 1.0
        cf16[0:1, O_ONES14 : O_ONES14 + 4] = 1.0

        cf32 = np.zeros((128, CF32), np.float32)
        CT = (4.0 * C_full.T).astype(np.float32)     # [H, N] (global rows)
        for t in range(HT):
            r = slice(t * 128, (t + 1) * 128)
            cf32[:, O_C + t * NM : O_C + (t + 1) * NM] = CT[r][:, i0 : i0 + NM]
            cf32[:, O_C3 + t * NP3 : O_C3 + (t + 1) * NP3] = CT[r][:, P3_ROWS]
            cf32[:, O_B2 + t] = b_e2[r]
            cf32[:, O_BN1 + t] = b_n1[r]
            cf32[:, O_BN2 + t] = b_n2[r]
            cf32[:, O_BO1 + t] = b_o1[r]
        cf32[0:D, O_BO] = b_o

        in_maps.append({
            "cf16": cf16,
            "cf32": cf32,
            "w28": w28,
            "adjr": adj[b, i0 : i0 + NM, :].astype(np.float16),
            "adj3": adj[b][P3_ROWS][:, half * NH : (half + 1) * NH].astype(np.float16),
        })
    return in_maps


def run(trace=False, **inputs):
    if "nc" not in _CACHE:
        _CACHE["nc"] = build_program()
    nc = _CACHE["nc"]
    in_maps = make_in_maps(**{k: np.asarray(v) for k, v in inputs.items()})
    r = run_bass_kernel_spmd(nc, in_maps, list(range(8)), trace=trace)
    out = np.empty((B, N, D), np.float32)
    for b in range(B):
        out[b, :NH] = r.results[2 * b]["out"].T
        out[b, NH:] = r.results[2 * b + 1]["out"].T
    return out, r


def kernel(**inputs):
    out, _ = run(trace=False, **inputs)
    return out


# revision 20
# speedup vs baseline: 1.0150x; 1.0150x over previous
"""GumbelGraphNetworkClf fused Bass kernel for 8 trn2 NeuronCores (raw bass).

Math (per batch b):
  pre[i,j,:] = x[j]@W_e1[:D] + x[i]@W_e1[D:] + b_e1   (= A[j] + C[i])
  n2e = relu(pre); e2e = relu(n2e @ W_e2 + b_e2)
  agg[j,:] = sum_i adj[i,j] * e2e[i,j,:]
  out = log_softmax(nodeMLP(agg, x), axis=-1)

Sharding: core c -> batch b = c//2, i-half = c%2.  Each core runs the edge
pipeline for its own i rows 0..239 over all 512 j (iterations 0..239), in
two accumulation phases (i 0..127, 128..239).  Each phase's partial agg is
pair-ReduceScattered over the j axis (rank = c%2 keeps its j-half); phase
1's collective hides under phase 2, phase 2's under a replicated tail
phase: both pair cores compute the 32 leftover global i rows ({240..255,
496..511}) for their own j-half at FD=256 (iterations 240..271).  The node
MLP + log_softmax then run on [*, 256] tiles and each core writes its half
of the output rows.

Per iteration it (fd = 512 main / 256 tail):
  DVE  n2e(it+4) = max(4A + 4C[:,it+4], 0) -> fp8     (2 tensor_scalar)
  PE   py(it)    = W2q.T @ n2e                        (2 fp8 DoubleRow MM)
       acc(it-9) agg += I @ msk(it-9)                 (2 f16 MM)
  ACT  e2e(it)   = relu(py/128 + b2) -> f16           (2 activation)
  DVE  msk quad  = e2e[4i block] * abc                (2 tensor_mul / 4 it)
W2 is sent as fp8(32*W2) and n2e is scaled by 4 into fp8's normal range
(the 1/128 comes out in the ACT scale), so the 256-deep contraction runs
as one DoubleRow matmul per output chunk.
"""

import sys

sys.path.insert(0, "/opt/trn_rl_repo")

import numpy as np
import ml_dtypes

import concourse.bass as bass
from concourse import mybir
from concourse.bass_utils import run_bass_kernel_spmd

B, N, D, H = 4, 512, 4, 256
NI = 256          # i rows per core's shard
NM = 240          # main-loop iterations (own i rows 0..239)
NP3 = 32          # replicated tail rows ({240..255, 496..511})
NT = NM + NP3     # total pipeline iterations
NH = N // 2       # j-half kept after ReduceScatter
HT = 2
LAG = 9           # acc(it-LAG) in PE iteration it
F16 = mybir.dt.float16
F32 = mybir.dt.float32
F8 = mybir.dt.float8e4
AF = mybir.ActivationFunctionType
OP = mybir.AluOpType
PM = mybir.MatmulPerfMode

# f16 const pack (cols) — A/A3/ident first (early DMA slice)
O_A = 0                    # 4*A^T halves          [128, 2N]
O_A3 = O_A + 2 * N         # 4*A^T j-half          [128, 2NH]
O_ID = O_A3 + 2 * NH       # identity              [128, 128]
NC16 = O_ID + 128          # end of early slice
O_WN1 = NC16               # W_n1 chunks           [128, 2H]
O_WN2 = O_WN1 + 2 * H
O_WO1H = O_WN2 + 2 * H
O_WO = O_WO1H + 2 * H      # W_o chunks            [128, 2D]
O_XT = O_WO + 2 * D        # x^T j-half            rows 0-3, NH cols
O_WO1X = O_XT + NH         # W_o1[:D]              rows 0-3, H cols
O_ONES4 = O_WO1X + H       # [4,1] ones
O_ONES14 = O_ONES4 + 1     # [1,4] ones
CF16 = O_ONES14 + 4
# f32 const pack (cols)
O_C = 0                    # 4*C^T own rows        [128, 2NM]
O_C3 = O_C + 2 * NM        # 4*C^T tail rows       [128, 2NP3]
O_B2 = O_C3 + 2 * NP3
O_BN1 = O_B2 + 2
O_BN2 = O_BN1 + 2
O_BO1 = O_BN2 + 2
O_BO = O_BO1 + 2           # rows 0-3
CF32 = O_BO + 1

_CACHE = {}


def _fd(it):
    return N if it < NM else NH


def _ph(k):
    return 0 if k < 128 else (1 if k < NM else 0)


def build_program():
    nc = bass.Bass("TRN2", target_bir_lowering=False, num_devices=8)

    cf16_ext = nc.dram_tensor("cf16", [128, CF16], F16, kind="ExternalInput")
    cf32_ext = nc.dram_tensor("cf32", [128, CF32], F32, kind="ExternalInput")
    w28_ext = nc.dram_tensor("w28", [128, 2, H], F8, kind="ExternalInput")
    adj_ext = nc.dram_tensor("adjr", [NM, N], F16, kind="ExternalInput")
    adj3_ext = nc.dram_tensor("adj3", [NP3, NH], F16, kind="ExternalInput")
    out_ext = nc.dram_tensor("out", [D, NH], F32, kind="ExternalOutput")
    aggd = [nc.dram_tensor(f"aggd{p}", [2, 128, 2, NH], F16) for p in range(2)]
    aggr = [nc.dram_tensor(f"aggr{p}", [128, 2, NH], F16) for p in range(2)]

    IT_COPY1 = 158      # ACT inserts phase-1 agg copies after this iteration
    IT_COPY2 = 246      # DVE inserts phase-2 agg copies after this iteration

    # ================= milestone bookkeeping (mirrors emission) =============
    # v_sem (DVE): prologue n2e(0..3); loop it: [n2e(it+4)], [quad at
    # it%4==3], [copy2 at IT_COPY2].
    vcnt = 0
    n2e_done, mskq_done_ho, mskq_done = {}, {}, {}
    V_COPY2 = None
    for k in range(4):
        vcnt += 2
        n2e_done[k] = vcnt
    for it in range(NT):
        if it + 4 < NT:
            vcnt += 2
            n2e_done[it + 4] = vcnt
        if it % 4 == 3:
            Q = it // 4
            vcnt += 1
            mskq_done_ho[(Q, 0)] = vcnt
            vcnt += 1
            mskq_done_ho[(Q, 1)] = vcnt
            mskq_done[Q] = vcnt
        if it == IT_COPY2:
            vcnt += 2
            V_COPY2 = vcnt

    # p_sem (PE): loop it: py(it) per ho (1 inc each); acc(it-LAG) (2 incs).
    pcnt = 0
    py_done_ho, py_done, acc_done = {}, {}, {}
    for it in range(NT + LAG):
        if it < NT:
            pcnt += 1
            py_done_ho[(it, 0)] = pcnt
            pcnt += 1
            py_done_ho[(it, 1)] = pcnt
            py_done[it] = pcnt
        if it >= LAG:
            pcnt += 2
            acc_done[it - LAG] = pcnt

    # a_sem (ACT): e2e per (it, ho); copy1 inserted after IT_COPY1.
    acnt = 0
    e2e_done_ho, e2e_done = {}, {}
    A_COPY1 = None
    for it in range(NT):
        for ho in range(HT):
            acnt += 1
            e2e_done_ho[(it, ho)] = acnt
        e2e_done[it] = acnt
        if it == IT_COPY1:
            acnt += 2
            A_COPY1 = acnt
    A_AGG3C = acnt + 2

    # d_sem: each dma incs 16.  Order: cf32, cf16-A, w28, cf16-A3/ident,
    # abc0, abc1, cf16rest, abc2..19, aggd1(4), abc20..22, abc3big,
    # abc23..29, aggd2(4), rs1 load, rs2 load, out.
    dcnt = 0
    def _dma():
        nonlocal dcnt
        dcnt += 16
        return dcnt
    for _ in range(3):
        D_CORE = _dma()
    abc_d = {}
    for k in range(2):
        abc_d[k] = _dma()
    D_REST = _dma()
    for k in range(2, 20):
        abc_d[k] = _dma()
    for _ in range(2):
        D_AGGD1 = _dma()
    for k in range(20, 23):
        abc_d[k] = _dma()
    D_ABC3 = _dma()
    for k in range(23, NM // 8):
        abc_d[k] = _dma()
    for _ in range(2):
        D_AGGD2 = _dma()
    D_R1 = _dma()
    D_R2 = _dma()

    def d_abc(it):
        return abc_d[it // 8] if it < NM else D_ABC3

    from contextlib import ExitStack
    with ExitStack() as ctx:
        e = ctx.enter_context
        cf16 = e(nc.sbuf_tensor([128, CF16], F16))
        cf32 = e(nc.sbuf_tensor([128, CF32], F32))
        w28 = e(nc.sbuf_tensor([128, 2, H], F8))
        abc = [e(nc.sbuf_tensor(f"abc{k}", [128, 8 * N], F16)) for k in range(2)]
        abc3 = e(nc.sbuf_tensor("abc3", [128, NP3 * NH], F16))
        n2e = [e(nc.sbuf_tensor(f"n2e{q}", [128, 2, N], F8)) for q in range(6)]
        e2eq = [[e(nc.sbuf_tensor(f"e2eq{t}{r}", [128, 4 * N], F16)) for r in range(3)] for t in range(2)]
        mskq = [[e(nc.sbuf_tensor(f"mskq{t}{r}", [128, 4 * N], F16)) for r in range(3)] for t in range(2)]
        aggf = [e(nc.sbuf_tensor(f"aggf{p}", [128, 2, N], F16)) for p in range(2)]
        agg2 = [e(nc.sbuf_tensor(f"agg2{p}", [128, 2, NH], F16)) for p in range(2)]
        agg3sb = [e(nc.sbuf_tensor(f"agg3sb{k}", [128, NH], F16)) for k in range(2)]
        out1 = [e(nc.sbuf_tensor(f"out1{k}", [128, NH], F16)) for k in range(2)]
        out2 = [e(nc.sbuf_tensor(f"out2{k}", [128, NH], F16)) for k in range(2)]
        out4 = [e(nc.sbuf_tensor(f"out4{k}", [128, NH], F16)) for k in range(2)]
        out5 = e(nc.sbuf_tensor([4, NH], F32))
        ex = e(nc.sbuf_tensor([4, NH], F16))
        ls = e(nc.sbuf_tensor([1, NH], F16))
        res = e(nc.sbuf_tensor([4, NH], F32))
        dumm = e(nc.sbuf_tensor([1, 2], F32))
        py00 = e(nc.psum_tensor([128, N], F32))
        py01 = e(nc.psum_tensor([128, N], F32))
        py10 = e(nc.psum_tensor([128, N], F32))
        py11 = e(nc.psum_tensor([128, N], F32))
        agg00 = e(nc.psum_tensor([128, N], F32))
        agg01 = e(nc.psum_tensor([128, N], F32))
        agg10 = e(nc.psum_tensor([128, N], F32))
        agg11 = e(nc.psum_tensor([128, N], F32))
        d_sem = e(nc.semaphore("d_sem"))
        v_sem = e(nc.semaphore("v_sem"))
        p_sem = e(nc.semaphore("p_sem"))
        a_sem = e(nc.semaphore("a_sem"))
        cc_sem = e(nc.semaphore("cc_sem"))
        pp_sem = e(nc.semaphore("pp_sem"))
        aa_sem = e(nc.semaphore("aa_sem"))
        vv_sem = e(nc.semaphore("vv_sem"))
        block = e(nc.Block())
        py = [[py00, py01], [py10, py11]]
        agg_ps = [[agg00, agg01], [agg10, agg11]]     # [phase][ho-chunk]
        A_sb = [cf16[:, O_A + t * N : O_A + (t + 1) * N] for t in range(HT)]
        A3_sb = [cf16[:, O_A3 + t * NH : O_A3 + (t + 1) * NH] for t in range(HT)]
        ident = cf16[:, O_ID : O_ID + 128]
        Wn1_sb = [cf16[:, O_WN1 + t * H : O_WN1 + (t + 1) * H] for t in range(HT)]
        Wn2_sb = [cf16[:, O_WN2 + t * H : O_WN2 + (t + 1) * H] for t in range(HT)]
        Wo1h_sb = [cf16[:, O_WO1H + t * H : O_WO1H + (t + 1) * H] for t in range(HT)]
        Wo_sb = [cf16[:, O_WO + t * D : O_WO + (t + 1) * D] for t in range(HT)]
        xT_sb = cf16[0:D, O_XT : O_XT + NH]
        Wo1x_sb = cf16[0:D, O_WO1X : O_WO1X + H]
        ones4 = cf16[0:D, O_ONES4 : O_ONES4 + 1]
        ones14 = cf16[0:1, O_ONES14 : O_ONES14 + 4]
        C_sb = [cf32[:, O_C + t * NM : O_C + (t + 1) * NM] for t in range(HT)]
        C3_sb = [cf32[:, O_C3 + t * NP3 : O_C3 + (t + 1) * NP3] for t in range(HT)]
        b2_sb = [cf32[:, O_B2 + t : O_B2 + t + 1] for t in range(HT)]
        bn1_sb = [cf32[:, O_BN1 + t : O_BN1 + t + 1] for t in range(HT)]
        bn2_sb = [cf32[:, O_BN2 + t : O_BN2 + t + 1] for t in range(HT)]
        bo1_sb = [cf32[:, O_BO1 + t : O_BO1 + t + 1] for t in range(HT)]
        bo_sb = cf32[0:D, O_BO : O_BO + 1]
        # post-loop PSUM reuse (agg10/11 free after copy2; py banks after loop)
        ps_mlp = [agg10[:, 0:NH], agg11[:, 0:NH]]
        ps5 = py00[0:4, 0:NH]
        ps_sum = py01[0:1, 0:NH]
        ps_ls4 = py11[0:4, 0:NH]

        def n2e_src(k, t):
            if k < NM:
                return A_sb[t], C_sb[t][:, k : k + 1]
            return A3_sb[t], C3_sb[t][:, k - NM : k - NM + 1]

        @block.sync
        def _(sync):
            sync.dma_start(cf32[:], cf32_ext[:, :]).then_inc(d_sem, 16)
            sync.dma_start(cf16[:, 0:NC16], cf16_ext[:, 0:NC16]).then_inc(d_sem, 16)
            sync.dma_start(w28[:, :, :], w28_ext[:, :, :]).then_inc(d_sem, 16)
            for k in range(NM // 8):
                if k >= 2:
                    sync.wait_ge(v_sem, mskq_done[2 * (k - 2) + 1])  # abc WAR
                sync.dma_start(
                    abc[k % 2][:],
                    adj_ext[None, 8 * k : 8 * (k + 1), :].broadcast_to([128, 8, N]),
                ).then_inc(d_sem, 16)
                if k == 1:
                    sync.dma_start(cf16[:, NC16:CF16], cf16_ext[:, NC16:CF16]).then_inc(d_sem, 16)
                if k == 19:
                    sync.wait_ge(a_sem, A_COPY1)
                    for half in range(2):
                        sync.dma_start(
                            aggd[0][half, :, :, :],
                            aggf[0][:, :, half * NH : (half + 1) * NH],
                        ).then_inc(d_sem, 16)
                if k == 22:
                    sync.dma_start(
                        abc3[:],
                        adj3_ext[None, :, :].broadcast_to([128, NP3, NH]),
                    ).then_inc(d_sem, 16)
            sync.wait_ge(v_sem, V_COPY2)
            for half in range(2):
                sync.dma_start(
                    aggd[1][half, :, :, :],
                    aggf[1][:, :, half * NH : (half + 1) * NH],
                ).then_inc(d_sem, 16)
            sync.wait_ge(cc_sem, 1)
            sync.dma_start(agg2[0][:, :, :], aggr[0][:, :, :]).then_inc(d_sem, 16)
            sync.wait_ge(cc_sem, 2)
            sync.dma_start(agg2[1][:, :, :], aggr[1][:, :, :]).then_inc(d_sem, 16)
            sync.wait_ge(vv_sem, 1)
            sync.dma_start(out_ext[:, :], res[:, :]).then_inc(d_sem, 16)

        @block.gpsimd
        def _(gpsimd):
            for p in range(2):
                gpsimd.wait_ge(d_sem, D_AGGD1 if p == 0 else D_AGGD2)
                nc.gpsimd.collective_compute(
                    "ReduceScatter", OP.add,
                    replica_groups=[[0, 1], [2, 3], [4, 5], [6, 7]],
                    ins=[aggd[p][:]], outs=[aggr[p][:]],
                ).then_inc(cc_sem, 1)

        @block.vector
        def _(vector):
            vector.wait_ge(d_sem, D_CORE)
            for k in range(4):
                for t in range(HT):
                    asrc, csrc = n2e_src(k, t)
                    nc.vector.tensor_scalar(
                        n2e[k % 6][:, t, 0:_fd(k)], asrc, csrc, 0.0,
                        op0=OP.add, op1=OP.max,
                    ).then_inc(v_sem, 1)
            for it in range(NT):
                if it + 4 < NT:
                    if it >= 2:
                        vector.wait_ge(p_sem, py_done[it - 2])  # n2e slot WAR
                    for t in range(HT):
                        asrc, csrc = n2e_src(it + 4, t)
                        nc.vector.tensor_scalar(
                            n2e[(it + 4) % 6][:, t, 0:_fd(it + 4)], asrc, csrc, 0.0,
                            op0=OP.add, op1=OP.max,
                        ).then_inc(v_sem, 1)
                if it % 4 == 3:
                    Q = it // 4
                    fd = _fd(it)
                    vector.wait_ge(d_sem, d_abc(it))
                    if Q >= 3:
                        vector.wait_ge(p_sem, acc_done[4 * (Q - 3) + 3])  # mskq WAR
                    if it < NM:
                        src1 = abc[(it // 8) % 2][:, (Q % 2) * 4 * N : (Q % 2 + 1) * 4 * N]
                    else:
                        q3 = Q - NM // 4
                        src1 = abc3[:, q3 * 4 * NH : (q3 + 1) * 4 * NH]
                    for ho in range(HT):
                        vector.wait_ge(a_sem, e2e_done_ho[(it, ho)])
                        nc.vector.tensor_mul(
                            mskq[ho][Q % 3][:, 0 : 4 * fd],
                            e2eq[ho][Q % 3][:, 0 : 4 * fd], src1,
                        ).then_inc(v_sem, 1)
                if it == IT_COPY2:
                    vector.wait_ge(p_sem, acc_done[NM - 1])
                    for t in range(HT):
                        nc.vector.tensor_copy(aggf[1][:, t, :], agg_ps[1][t][:]).then_inc(v_sem, 1)
            vector.wait_ge(pp_sem, 9)
            nc.vector.tensor_tensor(res[:], out5[:], ps_ls4, op=OP.subtract).then_inc(vv_sem, 1)

        @block.scalar
        def _(scalar):
            # activation-table preload: force the natural_log_exp set early
            nc.scalar.activation(dumm[:, 0:1], dumm[:, 1:2], AF.Ln)
            nc.scalar.activation(dumm[:, 0:1], dumm[:, 1:2], AF.Exp)
            for it in range(NT):
                fd = _fd(it)
                for ho in range(HT):
                    scalar.wait_ge(p_sem, py_done_ho[(it, ho)])
                    if it >= 12:
                        # e2eq[ho][(it//4)%3] slot WAR vs mskq of quad it//4-3
                        scalar.wait_ge(v_sem, mskq_done_ho[(it // 4 - 3, ho)])
                    nc.scalar.activation(
                        e2eq[ho][(it // 4) % 3][:, (it % 4) * fd : (it % 4 + 1) * fd],
                        py[ho][it % 2][:, 0:fd], AF.Relu, bias=b2_sb[ho], scale=0.0078125,
                    ).then_inc(a_sem, 1)
                if it == IT_COPY1:
                    scalar.wait_ge(p_sem, acc_done[127])
                    for t in range(HT):
                        nc.scalar.activation(aggf[0][:, t, :], agg_ps[0][t][:],
                                             AF.Identity).then_inc(a_sem, 1)
            scalar.wait_ge(p_sem, acc_done[NT - 1])
            for t in range(HT):
                nc.scalar.activation(agg3sb[t][:], agg_ps[0][t][:, 0:NH],
                                     AF.Identity).then_inc(a_sem, 1)
            # ---- node MLP activations ----
            for ho in range(HT):
                scalar.wait_ge(pp_sem, ho + 1)
                nc.scalar.activation(out1[ho][:], ps_mlp[ho], AF.Relu,
                                     bias=bn1_sb[ho]).then_inc(aa_sem, 1)
            for ho in range(HT):
                scalar.wait_ge(pp_sem, 2 + ho + 1)
                nc.scalar.activation(out2[ho][:], ps_mlp[ho], AF.Relu,
                                     bias=bn2_sb[ho]).then_inc(aa_sem, 1)
            for ho in range(HT):
                scalar.wait_ge(pp_sem, 4 + ho + 1)
                nc.scalar.activation(out4[ho][:], ps_mlp[ho], AF.Identity,
                                     bias=bo1_sb[ho]).then_inc(aa_sem, 1)
            scalar.wait_ge(pp_sem, 7)
            nc.scalar.activation(ex[:], ps5, AF.Exp, bias=bo_sb).then_inc(aa_sem, 1)
            nc.scalar.activation(out5[:], ps5, AF.Identity, bias=bo_sb).then_inc(aa_sem, 1)
            scalar.wait_ge(pp_sem, 8)
            nc.scalar.activation(ls[:], ps_sum, AF.Ln).then_inc(aa_sem, 1)

        @block.tensor
        def _(pe):
            for it in range(NT + LAG):
                if it < NT:
                    fd = _fd(it)
                    q = it % 2
                    pe.wait_ge(v_sem, n2e_done[it])
                    for ho in range(HT):
                        if it >= 2:
                            pe.wait_ge(a_sem, e2e_done_ho[(it - 2, ho)])  # py WAR
                        hs = slice(ho * 128, (ho + 1) * 128)
                        nc.tensor.matmul(
                            py[ho][q][:, 0:fd], w28[:, 0:2, hs], n2e[it % 6][:, :, 0:fd],
                            start=True, stop=True, perf_mode=PM.DoubleRow,
                        ).then_inc(p_sem, 1)
                if it >= LAG:
                    k = it - LAG
                    fdk = _fd(k)
                    ph = _ph(k)
                    pe.wait_ge(v_sem, mskq_done[k // 4])
                    for ho in range(HT):
                        nc.tensor.matmul(
                            agg_ps[ph][ho][:, 0:fdk], ident,
                            mskq[ho][(k // 4) % 3][:, (k % 4) * fdk : (k % 4 + 1) * fdk],
                            start=(k in (0, 128, NM)), stop=(k in (127, NM - 1, NT - 1)),
                        ).then_inc(p_sem, 1)
            # ---- node MLP matmuls ----  pp milestones:
            # 1,2: mlp1[ho]  3,4: mlp2[ho]  5,6: out4 ps[ho]  7: ps5
            # 8: ps_sum  9: ps_ls4
            pe.wait_ge(d_sem, D_REST)
            pe.wait_ge(v_sem, V_COPY2)
            pe.wait_ge(d_sem, D_R1)
            for ho in range(HT):
                for t in range(HT):
                    nc.tensor.matmul(
                        ps_mlp[ho], Wn1_sb[t][:, ho * 128 : (ho + 1) * 128],
                        agg2[0][:, t, :], start=(t == 0), stop=False,
                    )
            pe.wait_ge(a_sem, A_AGG3C)
            for ho in range(HT):
                for t in range(HT):
                    nc.tensor.matmul(
                        ps_mlp[ho], Wn1_sb[t][:, ho * 128 : (ho + 1) * 128],
                        agg3sb[t][:], start=False, stop=False,
                    )
            pe.wait_ge(d_sem, D_R2)
            for ho in range(HT):
                for t in range(HT):
                    mm = nc.tensor.matmul(
                        ps_mlp[ho], Wn1_sb[t][:, ho * 128 : (ho + 1) * 128],
                        agg2[1][:, t, :], start=False, stop=(t == 1),
                    )
                mm.then_inc(pp_sem, 1)
            for ho in range(HT):
                pe.wait_ge(aa_sem, 2)
                for t in range(HT):
                    mm = nc.tensor.matmul(
                        ps_mlp[ho], Wn2_sb[t][:, ho * 128 : (ho + 1) * 128],
                        out1[t][:], start=(t == 0), stop=(t == 1),
                    )
                mm.then_inc(pp_sem, 1)
            for ho in range(HT):
                pe.wait_ge(aa_sem, 4)
                nc.tensor.matmul(
                    ps_mlp[ho], Wo1x_sb[:, ho * 128 : (ho + 1) * 128], xT_sb,
                    start=True, stop=False,
                )
                for t in range(HT):
                    mm = nc.tensor.matmul(
                        ps_mlp[ho], Wo1h_sb[t][:, ho * 128 : (ho + 1) * 128],
                        out2[t][:], start=False, stop=(t == 1),
                    )
                mm.then_inc(pp_sem, 1)
            pe.wait_ge(aa_sem, 6)
            for t in range(HT):
                mm = nc.tensor.matmul(
                    ps5, Wo_sb[t], out4[t][:], start=(t == 0), stop=(t == 1),
                )
            mm.then_inc(pp_sem, 1)
            pe.wait_ge(aa_sem, 7)                             # ex ready
            nc.tensor.matmul(ps_sum, ones4, ex[:], start=True, stop=True).then_inc(pp_sem, 1)
            pe.wait_ge(aa_sem, 9)                             # ls ready
            nc.tensor.matmul(ps_ls4, ones14, ls[:], start=True, stop=True).then_inc(pp_sem, 1)

    return nc


P3_ROWS = list(range(NM, NI)) + list(range(NI + NM, 2 * NI))


def make_in_maps(x, adj, W_e1, b_e1, W_e2, b_e2, W_n1, b_n1, W_n2, b_n2,
                 W_o1, b_o1, W_o, b_o):
    F8NP = ml_dtypes.float8_e4m3
    w28 = np.zeros((128, 2, H), F8NP)
    w2s = 32.0 * W_e2.astype(np.float32)
    for t in range(HT):
        w28[:, t, :] = w2s[t * 128 : (t + 1) * 128].astype(F8NP)

    in_maps = []
    for c in range(8):
        b = c // 2
        half = c % 2
        i0 = half * NI
        A_full = x[b] @ W_e1[:D]                     # [N, H]
        C_full = x[b] @ W_e1[D:] + b_e1              # [N, H]

        cf16 = np.zeros((128, CF16), np.float16)
        AT = (4.0 * A_full.T).astype(np.float16)     # [H, N]
        for t in range(HT):
            r = slice(t * 128, (t + 1) * 128)
            cf16[:, O_A + t * N : O_A + (t + 1) * N] = AT[r]
            cf16[:, O_A3 + t * NH : O_A3 + (t + 1) * NH] = AT[r][:, half * NH : (half + 1) * NH]
            cf16[:, O_WN1 + t * H : O_WN1 + (t + 1) * H] = W_n1[r].astype(np.float16)
            cf16[:, O_WN2 + t * H : O_WN2 + (t + 1) * H] = W_n2[r].astype(np.float16)
            cf16[:, O_WO1H + t * H : O_WO1H + (t + 1) * H] = W_o1[D:][r].astype(np.float16)
            cf16[:, O_WO + t * D : O_WO + (t + 1) * D] = W_o[r].astype(np.float16)
        cf16[:, O_ID : O_ID + 128] = np.eye(128, dtype=np.float16)
        cf16[0:D, O_XT : O_XT + NH] = x[b].T[:, half * NH : (half + 1) * NH]
        cf16[0:D, O_WO1X : O_WO1X + H] = W_o1[:D]
        cf16[0:D, O_ONES4] = 1.0
        cf16[0:1, O_ONES14 : O_ONES14 + 4] = 1.0

        cf32 = np.zeros((128, CF32), np.float32)
        CT = (4.0 * C_full.T).astype(np.float32)     # [H, N] (global rows)
        for t in range(HT):
            r = slice(t * 128, (t + 1) * 128)
            cf32[:, O_C + t * NM : O_C + (t + 1) * NM] = CT[r][:, i0 : i0 + NM]
            cf32[:, O_C3 + t * NP3 : O_C3 + (t + 1) * NP3] = CT[r][:, P3_ROWS]
            cf32[:, O_B2 + t] = b_e2[r]
            cf32[:, O_BN1 + t] = b_n1[r]
            cf32[:, O_BN2 + t] = b_n2[r]
            cf32[:, O_BO1 + t] = b_o1[r]
        cf32[0:D, O_BO] = b_o

        in_maps.append({
            "cf16": cf16,
            "cf32": cf32,
            "w28": w28,
            "adjr": adj[b, i0 : i0 + NM, :].astype(np.float16),
            "adj3": adj[b][P3_ROWS][:, half * NH : (half + 1) * NH].astype(np.float16),
        })
    return in_maps


def run(trace=False, **inputs):
    if "nc" not in _CACHE:
        _CACHE["nc"] = build_program()
    nc = _CACHE["nc"]
    in_maps = make_in_maps(**{k: np.asarray(v) for k, v in inputs.items()})
    r = run_bass_kernel_spmd(nc, in_maps, list(range(8)), trace=trace)
    out = np.empty((B, N, D), np.float32)
    for b in range(B):
        out[b, :NH] = r.results[2 * b]["out"].T
        out[b, NH:] = r.results[2 * b + 1]["out"].T
    return out, r


def kernel(**inputs):
    out, _ = run(trace=False, **inputs)
    return out
